# revision 12
# baseline (speedup 1.0000x reference)
"""DDRF mixer kernel for 8 trn2 NeuronCores.

out[b,t,:] = sum_n softmax_n(x[b,t,:] @ W.T)[n] * x[b, t - o_n, :],
offsets o = (1,2,4,8,16,32,64), x: [4,4096,1024] f32, W: [7,1024] f32.

Sharding: 8 shards of 2048 tokens (batch b = c//2, half h = c%2), each with a
128-token halo tile prepended (zeros at sequence start).  Per 128-token tile:
  - logits via 8 PE transposes of the x tile (bf16) + 8 accumulating matmuls
    against W.T chunks -> PSUM [128,7]
  - softmax on-chip (fp32)
  - the tap-weighted gather is ONE banded matrix multiply: A[t, j'] holds
    w[t,n] at j' = t + 64 - o_n (built with 7 masked tensor_scalar ops from
    constant diagonal masks), transposed on PE, then out = A_lo.T@x_prev +
    A_hi.T@x_cur accumulated in PSUM.
Compute dtype bf16 (fp32 PSUM accumulation); measured rel err vs fp32
reference ~2.7e-3.
"""

import sys

if "/opt/trn_rl_repo" not in sys.path:
    sys.path.insert(0, "/opt/trn_rl_repo")

import numpy as np
import ml_dtypes

BF16 = ml_dtypes.bfloat16

OFFSETS = (1, 2, 4, 8, 16, 32, 64)
NTAPS = 7
B, T, D = 4, 4096, 1024
NCORES = 8
CHUNK = 2048          # output tokens per core
TT = 128              # token tile (partition dim)
NTILES = CHUNK // TT  # 16
XS_TOKENS = CHUNK + TT
DCH = D // 128        # 8 d-chunks
JW = 192              # A width: j' = j - 64, j in [64, 256)
NSPLIT = 512          # matmul moving-operand split (PSUM bank)

_prog_cache = {}


def _build_program():
    from contextlib import ExitStack
    import concourse.bass as bass  # noqa: F401
    import concourse.tile as tile
    from concourse import bacc, mybir

    f32 = mybir.dt.float32
    bf16 = mybir.dt.bfloat16
    AX = mybir.AxisListType.X
    OP = mybir.AluOpType
    AF = mybir.ActivationFunctionType

    nc = bacc.Bacc(
        "TRN2", target_bir_lowering=False, debug=False, num_devices=NCORES
    )
    xs = nc.dram_tensor("xs", [XS_TOKENS, D], bf16, kind="ExternalInput").ap()
    # wt[p, c, n] = W.T[p*DCH + c, n] — matches the xbar-transposed x layout
    wt = nc.dram_tensor("wt", [128, DCH, NTAPS], bf16, kind="ExternalInput").ap()
    mk = nc.dram_tensor("mk", [NTAPS, TT, JW], bf16, kind="ExternalInput").ap()
    idn = nc.dram_tensor("idn", [128, 128], bf16, kind="ExternalInput").ap()
    out = nc.dram_tensor("out", [CHUNK, D], f32, kind="ExternalOutput").ap()

    with tile.TileContext(nc) as tc:
        with ExitStack() as ctx:
            const = ctx.enter_context(tc.tile_pool(name="const", bufs=1))
            xpool = ctx.enter_context(tc.tile_pool(name="xp", bufs=6))
            xts = ctx.enter_context(tc.tile_pool(name="xts", bufs=3))
            lgp = ctx.enter_context(tc.tile_pool(name="lgp", bufs=2, space="PSUM"))
            smp = ctx.enter_context(tc.tile_pool(name="smp", bufs=3))
            apl = ctx.enter_context(tc.tile_pool(name="apl", bufs=3))
            atp = ctx.enter_context(tc.tile_pool(name="atp", bufs=2, space="PSUM"))
            ats = ctx.enter_context(tc.tile_pool(name="ats", bufs=3))
            outp = ctx.enter_context(tc.tile_pool(name="outp", bufs=2, space="PSUM"))
            outs = ctx.enter_context(tc.tile_pool(name="outs", bufs=3))

            wt_sb = const.tile([128, DCH, NTAPS], bf16)
            nc.sync.dma_start(wt_sb[:], wt[:, :, :])
            mk_sb = const.tile([128, NTAPS, JW], bf16)
            nc.sync.dma_start(mk_sb[:], mk.rearrange("n t j -> t n j"))
            id_sb = const.tile([128, 128], bf16)
            nc.sync.dma_start(id_sb[:], idn[:, :])

            xt = []

            def load_x(k):
                t = xpool.tile([TT, D], bf16, tag="x")
                nc.sync.dma_start(t[:], xs[k * TT:(k + 1) * TT, :])
                xt.append(t)

            load_x(0)
            load_x(1)

            for i in range(NTILES):
                if i + 2 <= NTILES:
                    load_x(i + 2)
                xp_t = xt[i]       # prev tile (halo for i=0)
                xc_t = xt[i + 1]   # current tile

                # -- logits: xbar-transpose x (sxt[p, c, t] = x[t, c*128+p])
                #    on the scalar HWDGE queue (separate from sync's copies),
                #    then 8 accumulating matmuls against matching W.T chunks --
                sxt = xts.tile([128, DCH, 128], bf16, tag="sxt")
                nc.scalar.dma_start_transpose(sxt[:], xc_t[:])
                lg = lgp.tile([128, NTAPS], f32, tag="lg")
                for k in range(DCH):
                    nc.tensor.matmul(
                        lg[:],
                        lhsT=sxt[:, k, :],
                        rhs=wt_sb[:, k, :],
                        start=(k == 0),
                        stop=(k == DCH - 1),
                    )

                # -- softmax over taps (free dim, 7 wide); logits are small
                #    (|z| <~ 5), so skip the max-subtraction --
                esb = smp.tile([128, NTAPS], f32, tag="esb")
                ssum = smp.tile([128, 1], f32, tag="ssum")
                nc.scalar.activation(
                    esb[:], lg[:], AF.Exp,
                    bias=0.0, scale=1.0, accum_out=ssum[:, 0:1],
                )
                rec = smp.tile([128, 1], f32, tag="rec")
                nc.vector.reciprocal(rec[:], ssum[:])
                wsb = smp.tile([128, NTAPS], f32, tag="wsb")
                nc.vector.tensor_scalar(
                    out=wsb[:], in0=esb[:], scalar1=rec[:, 0:1], scalar2=None,
                    op0=OP.mult,
                )

                # -- banded A build: A[t, t + 64 - o_n] = w[t, n] --
                # init with the widest tap (o=64, band [0,128), zero elsewhere)
                # over the full width, then add the other taps on their
                # 128-wide bands [64-o, 192-o).
                a_t = apl.tile([128, JW], bf16, tag="a")
                nc.vector.tensor_scalar(
                    out=a_t[:], in0=mk_sb[:, 6, :], scalar1=wsb[:, 6:7],
                    scalar2=None, op0=OP.mult,
                )
                for n in range(NTAPS - 1):
                    o = OFFSETS[n]
                    bl, bh = 64 - o, 192 - o
                    nc.vector.scalar_tensor_tensor(
                        out=a_t[:, bl:bh], in0=mk_sb[:, n, bl:bh],
                        scalar=wsb[:, n:n + 1],
                        in1=a_t[:, bl:bh], op0=OP.mult, op1=OP.add,
                    )

                # -- A -> A.T pieces --
                # LO piece lives at partitions [64,128) so its base partition
                # matches xp_t[64:128] in the matmul below.
                pat = atp.tile([128, 2, 128], bf16, tag="pat")
                nc.tensor.transpose(pat[64:128, 0, :], a_t[:, 0:64], id_sb[:])
                nc.tensor.transpose(pat[:, 1, :], a_t[:, 64:JW], id_sb[:])
                sat = ats.tile([128, 2, 128], bf16, tag="sat")
                nc.vector.tensor_copy(sat[64:128, 0, :], pat[64:128, 0, :])
                nc.vector.tensor_copy(sat[:, 1, :], pat[:, 1, :])

                # -- tap-weighted gather as one banded matmul --
                po = outp.tile([128, D], f32, tag="po")
                for h in range(D // NSPLIT):
                    cs = slice(h * NSPLIT, (h + 1) * NSPLIT)
                    nc.tensor.matmul(
                        po[:, cs], lhsT=sat[64:128, 0, :], rhs=xp_t[64:128, cs],
                        start=True, stop=False,
                    )
                    nc.tensor.matmul(
                        po[:, cs], lhsT=sat[:, 1, :], rhs=xc_t[:, cs],
                        start=False, stop=True,
                    )
                ob = outs.tile([128, D], f32, tag="ob")
                nc.scalar.copy(ob[:, 0:NSPLIT], po[:, 0:NSPLIT])
                nc.vector.tensor_copy(ob[:, NSPLIT:D], po[:, NSPLIT:D])
                nc.sync.dma_start(out[i * TT:(i + 1) * TT, :], ob[:])

    nc.compile()
    return nc


def _get_program():
    if "nc" not in _prog_cache:
        _prog_cache["nc"] = _build_program()
    return _prog_cache["nc"]


def _host_inputs(x, W):
    xb = np.asarray(x).astype(BF16)
    # [1024, 7] -> [128, 8, 7]: wt[p, c, n] = W.T[c*128+p, n], matching the
    # xbar-transposed x layout (sxt[p, c, t] = x[t, c*128+p])
    wtb = np.ascontiguousarray(
        np.asarray(W, dtype=np.float32).T
        .reshape(DCH, 128, NTAPS).transpose(1, 0, 2)
    ).astype(BF16)
    mk = np.zeros((NTAPS, TT, JW), np.float32)
    for n, o in enumerate(OFFSETS):
        for t in range(TT):
            mk[n, t, t + 64 - o] = 1.0
    mk = mk.astype(BF16)
    idn = np.eye(128, dtype=np.float32).astype(BF16)
    in_maps = []
    for c in range(NCORES):
        b, h = divmod(c, 2)
        t0 = h * CHUNK
        if h == 0:
            halo = np.zeros((TT, D), BF16)
        else:
            halo = xb[b, t0 - TT:t0]
        xs = np.ascontiguousarray(
            np.concatenate([halo, xb[b, t0:t0 + CHUNK]], axis=0)
        )
        in_maps.append({"xs": xs, "wt": wtb, "mk": mk, "idn": idn})
    return in_maps


def kernel(x, W, _trace=False):
    from concourse.bass_utils import run_bass_kernel_spmd

    nc = _get_program()
    in_maps = _host_inputs(x, W)
    res = run_bass_kernel_spmd(nc, in_maps, list(range(NCORES)), trace=_trace)
    out_full = np.empty((B, T, D), np.float32)
    for c in range(NCORES):
        b, h = divmod(c, 2)
        out_full[b, h * CHUNK:(h + 1) * CHUNK] = res.results[c]["out"]
    kernel.last_results = res
    return out_full


# revision 15
# speedup vs baseline: 1.4693x; 1.4693x over previous
"""DDRF mixer kernel for 8 trn2 NeuronCores.

out[b,t,:] = sum_n softmax_n(x[b,t,:] @ W.T)[n] * x[b, t - o_n, :],
offsets o = (1,2,4,8,16,32,64), x: [4,4096,1024] f32, W: [7,1024] f32.

Sharding: 8 shards of 2048 tokens (batch b = c//2, half h = c%2), each with a
128-token halo tile prepended (zeros at sequence start).  Per 128-token tile:
  - logits via 8 PE transposes of the x tile (bf16) + 8 accumulating matmuls
    against W.T chunks -> PSUM [128,7]
  - softmax on-chip (fp32)
  - the tap-weighted gather is ONE banded matrix multiply: A[t, j'] holds
    w[t,n] at j' = t + 64 - o_n (built with 7 masked tensor_scalar ops from
    constant diagonal masks), transposed on PE, then out = A_lo.T@x_prev +
    A_hi.T@x_cur accumulated in PSUM.
Compute dtype bf16 (fp32 PSUM accumulation); measured rel err vs fp32
reference ~2.7e-3.
"""

import sys

if "/opt/trn_rl_repo" not in sys.path:
    sys.path.insert(0, "/opt/trn_rl_repo")

import numpy as np
import ml_dtypes

BF16 = ml_dtypes.bfloat16

OFFSETS = (1, 2, 4, 8, 16, 32, 64)
NTAPS = 7
B, T, D = 4, 4096, 1024
NCORES = 8
CHUNK = 2048          # output tokens per core
TT = 128              # token tile (partition dim)
NTILES = CHUNK // TT  # 16
XS_TOKENS = CHUNK + TT
DCH = D // 128        # 8 d-chunks
JW = 192              # A width: j' = j - 64, j in [64, 256)
NSPLIT = 512          # matmul moving-operand split (PSUM bank)

_prog_cache = {}


def _build_program():
    from contextlib import ExitStack
    import concourse.bass as bass  # noqa: F401
    import concourse.tile as tile
    from concourse import bacc, mybir

    f32 = mybir.dt.float32
    bf16 = mybir.dt.bfloat16
    AX = mybir.AxisListType.X
    OP = mybir.AluOpType
    AF = mybir.ActivationFunctionType

    nc = bacc.Bacc(
        "TRN2", target_bir_lowering=False, debug=False, num_devices=NCORES
    )
    xs = nc.dram_tensor("xs", [XS_TOKENS, D], bf16, kind="ExternalInput").ap()
    # wt[p, c, n] = W.T[p*DCH + c, n] — matches the xbar-transposed x layout
    wt = nc.dram_tensor("wt", [128, DCH, NTAPS], bf16, kind="ExternalInput").ap()
    mk = nc.dram_tensor("mk", [NTAPS, TT, JW], bf16, kind="ExternalInput").ap()
    idn = nc.dram_tensor("idn", [128, 128], bf16, kind="ExternalInput").ap()
    out = nc.dram_tensor("out", [CHUNK, D], f32, kind="ExternalOutput").ap()

    with tile.TileContext(nc) as tc:
        with ExitStack() as ctx:
            const = ctx.enter_context(tc.tile_pool(name="const", bufs=1))
            xpool = ctx.enter_context(tc.tile_pool(name="xp", bufs=6))
            xtp = ctx.enter_context(tc.tile_pool(name="xtp", bufs=2, space="PSUM"))
            xts = ctx.enter_context(tc.tile_pool(name="xts", bufs=3))
            lgp = ctx.enter_context(tc.tile_pool(name="lgp", bufs=1, space="PSUM"))
            smp = ctx.enter_context(tc.tile_pool(name="smp", bufs=3))
            apl = ctx.enter_context(tc.tile_pool(name="apl", bufs=3))
            atp = ctx.enter_context(tc.tile_pool(name="atp", bufs=1, space="PSUM"))
            ats = ctx.enter_context(tc.tile_pool(name="ats", bufs=3))
            outp = ctx.enter_context(tc.tile_pool(name="outp", bufs=2, space="PSUM"))
            outs = ctx.enter_context(tc.tile_pool(name="outs", bufs=3))

            wt_sb = const.tile([128, DCH, NTAPS], bf16)
            nc.sync.dma_start(wt_sb[:], wt[:, :, :])
            mk_sb = const.tile([128, NTAPS, JW], bf16)
            nc.sync.dma_start(mk_sb[:], mk.rearrange("n t j -> t n j"))
            id_sb = const.tile([128, 128], bf16)
            nc.sync.dma_start(id_sb[:], idn[:, :])

            xt = []

            def load_x(k):
                t = xpool.tile([TT, D], bf16, tag="x")
                nc.sync.dma_start(t[:], xs[k * TT:(k + 1) * TT, :])
                xt.append(t)

            load_x(0)
            load_x(1)

            for i in range(NTILES):
                if i + 2 <= NTILES:
                    load_x(i + 2)
                xp_t = xt[i]       # prev tile (halo for i=0)
                xc_t = xt[i + 1]   # current tile

                # -- logits: PE-transpose x into PSUM (pxt[p, c, t] =
                #    x[t, c*128+p]), one bulk copy to SBUF, then 8
                #    accumulating matmuls against matching W.T chunks --
                pxt = xtp.tile([128, DCH, 128], bf16, tag="pxt")
                for k in range(DCH):
                    nc.tensor.transpose(
                        pxt[:, k, :], xc_t[:, k * 128:(k + 1) * 128], id_sb[:]
                    )
                sxt = xts.tile([128, DCH, 128], bf16, tag="sxt")
                nc.scalar.copy(sxt[:], pxt[:])
                lg = lgp.tile([128, NTAPS], f32, tag="lg")
                for k in range(DCH):
                    nc.tensor.matmul(
                        lg[:],
                        lhsT=sxt[:, k, :],
                        rhs=wt_sb[:, k, :],
                        start=(k == 0),
                        stop=(k == DCH - 1),
                    )

                # -- softmax over taps (free dim, 7 wide); logits are small
                #    (|z| <~ 5), so skip the max-subtraction --
                esb = smp.tile([128, NTAPS], f32, tag="esb")
                ssum = smp.tile([128, 1], f32, tag="ssum")
                nc.scalar.activation(
                    esb[:], lg[:], AF.Exp,
                    bias=0.0, scale=1.0, accum_out=ssum[:, 0:1],
                )
                rec = smp.tile([128, 1], f32, tag="rec")
                nc.vector.reciprocal(rec[:], ssum[:])
                wsb = smp.tile([128, NTAPS], f32, tag="wsb")
                nc.vector.tensor_scalar(
                    out=wsb[:], in0=esb[:], scalar1=rec[:, 0:1], scalar2=None,
                    op0=OP.mult,
                )

                # -- banded A build: A[t, t + 64 - o_n] = w[t, n] --
                # init with the widest tap (o=64, band [0,128), zero elsewhere)
                # over the full width, then add the other taps on their
                # 128-wide bands [64-o, 192-o).
                a_t = apl.tile([128, JW], bf16, tag="a")
                nc.vector.tensor_scalar(
                    out=a_t[:], in0=mk_sb[:, 6, :], scalar1=wsb[:, 6:7],
                    scalar2=None, op0=OP.mult,
                )
                for n in range(NTAPS - 1):
                    o = OFFSETS[n]
                    bl, bh = 64 - o, 192 - o
                    nc.vector.scalar_tensor_tensor(
                        out=a_t[:, bl:bh], in0=mk_sb[:, n, bl:bh],
                        scalar=wsb[:, n:n + 1],
                        in1=a_t[:, bl:bh], op0=OP.mult, op1=OP.add,
                    )

                # -- A -> A.T pieces --
                # LO piece lives at partitions [64,128) so its base partition
                # matches xp_t[64:128] in the matmul below.
                pat = atp.tile([128, 2, 128], bf16, tag="pat")
                nc.tensor.transpose(pat[64:128, 0, :], a_t[:, 0:64], id_sb[:])
                nc.tensor.transpose(pat[:, 1, :], a_t[:, 64:JW], id_sb[:])
                sat = ats.tile([128, 2, 128], bf16, tag="sat")
                nc.vector.tensor_copy(sat[64:128, 0, :], pat[64:128, 0, :])
                nc.vector.tensor_copy(sat[:, 1, :], pat[:, 1, :])

                # -- tap-weighted gather as one banded matmul --
                po = outp.tile([128, D], f32, tag="po")
                for h in range(D // NSPLIT):
                    cs = slice(h * NSPLIT, (h + 1) * NSPLIT)
                    nc.tensor.matmul(
                        po[:, cs], lhsT=sat[64:128, 0, :], rhs=xp_t[64:128, cs],
                        start=True, stop=False,
                    )
                    nc.tensor.matmul(
                        po[:, cs], lhsT=sat[:, 1, :], rhs=xc_t[:, cs],
                        start=False, stop=True,
                    )
                ob = outs.tile([128, D], f32, tag="ob")
                nc.scalar.copy(ob[:, 0:NSPLIT], po[:, 0:NSPLIT])
                nc.vector.tensor_copy(ob[:, NSPLIT:D], po[:, NSPLIT:D])
                nc.sync.dma_start(out[i * TT:(i + 1) * TT, :], ob[:])

    nc.compile()
    return nc


def _get_program():
    if "nc" not in _prog_cache:
        _prog_cache["nc"] = _build_program()
    return _prog_cache["nc"]


def _host_inputs(x, W):
    xb = np.asarray(x).astype(BF16)
    # [1024, 7] -> [128, 8, 7]: wt[p, c, n] = W.T[c*128+p, n], matching the
    # xbar-transposed x layout (sxt[p, c, t] = x[t, c*128+p])
    wtb = np.ascontiguousarray(
        np.asarray(W, dtype=np.float32).T
        .reshape(DCH, 128, NTAPS).transpose(1, 0, 2)
    ).astype(BF16)
    mk = np.zeros((NTAPS, TT, JW), np.float32)
    for n, o in enumerate(OFFSETS):
        for t in range(TT):
            mk[n, t, t + 64 - o] = 1.0
    mk = mk.astype(BF16)
    idn = np.eye(128, dtype=np.float32).astype(BF16)
    in_maps = []
    for c in range(NCORES):
        b, h = divmod(c, 2)
        t0 = h * CHUNK
        if h == 0:
            halo = np.zeros((TT, D), BF16)
        else:
            halo = xb[b, t0 - TT:t0]
        xs = np.ascontiguousarray(
            np.concatenate([halo, xb[b, t0:t0 + CHUNK]], axis=0)
        )
        in_maps.append({"xs": xs, "wt": wtb, "mk": mk, "idn": idn})
    return in_maps


def kernel(x, W, _trace=False):
    from concourse.bass_utils import run_bass_kernel_spmd

    nc = _get_program()
    in_maps = _host_inputs(x, W)
    res = run_bass_kernel_spmd(nc, in_maps, list(range(NCORES)), trace=_trace)
    out_full = np.empty((B, T, D), np.float32)
    for c in range(NCORES):
        b, h = divmod(c, 2)
        out_full[b, h * CHUNK:(h + 1) * CHUNK] = res.results[c]["out"]
    kernel.last_results = res
    return out_full


# revision 16
# speedup vs baseline: 1.5343x; 1.0442x over previous
"""DDRF mixer kernel for 8 trn2 NeuronCores.

out[b,t,:] = sum_n softmax_n(x[b,t,:] @ W.T)[n] * x[b, t - o_n, :],
offsets o = (1,2,4,8,16,32,64), x: [4,4096,1024] f32, W: [7,1024] f32.

Sharding: 8 shards of 2048 tokens (batch b = c//2, half h = c%2), each with a
128-token halo tile prepended (zeros at sequence start).  Per 128-token tile:
  - logits via 8 PE transposes of the x tile (bf16) + 8 accumulating matmuls
    against W.T chunks -> PSUM [128,7]
  - softmax on-chip (fp32)
  - the tap-weighted gather is ONE banded matrix multiply: A[t, j'] holds
    w[t,n] at j' = t + 64 - o_n (built with 7 masked tensor_scalar ops from
    constant diagonal masks), transposed on PE, then out = A_lo.T@x_prev +
    A_hi.T@x_cur accumulated in PSUM.
Compute dtype bf16 (fp32 PSUM accumulation); measured rel err vs fp32
reference ~2.7e-3.
"""

import sys

if "/opt/trn_rl_repo" not in sys.path:
    sys.path.insert(0, "/opt/trn_rl_repo")

import numpy as np
import ml_dtypes

BF16 = ml_dtypes.bfloat16

OFFSETS = (1, 2, 4, 8, 16, 32, 64)
NTAPS = 7
B, T, D = 4, 4096, 1024
NCORES = 8
CHUNK = 2048          # output tokens per core
TT = 128              # token tile (partition dim)
NTILES = CHUNK // TT  # 16
XS_TOKENS = CHUNK + TT
DCH = D // 128        # 8 d-chunks
JW = 192              # A width: j' = j - 64, j in [64, 256)
NSPLIT = 512          # matmul moving-operand split (PSUM bank)

_prog_cache = {}


def _build_program():
    from contextlib import ExitStack
    import concourse.bass as bass  # noqa: F401
    import concourse.tile as tile
    from concourse import bacc, mybir

    f32 = mybir.dt.float32
    bf16 = mybir.dt.bfloat16
    AX = mybir.AxisListType.X
    OP = mybir.AluOpType
    AF = mybir.ActivationFunctionType

    nc = bacc.Bacc(
        "TRN2", target_bir_lowering=False, debug=False, num_devices=NCORES
    )
    xs = nc.dram_tensor("xs", [XS_TOKENS, D], bf16, kind="ExternalInput").ap()
    # wt[p, c, n] = W.T[p*DCH + c, n] — matches the xbar-transposed x layout
    wt = nc.dram_tensor("wt", [128, DCH, NTAPS], bf16, kind="ExternalInput").ap()
    mk = nc.dram_tensor("mk", [NTAPS, TT, JW], bf16, kind="ExternalInput").ap()
    idn = nc.dram_tensor("idn", [128, 128], bf16, kind="ExternalInput").ap()
    out = nc.dram_tensor("out", [CHUNK, D], f32, kind="ExternalOutput").ap()

    with tile.TileContext(nc) as tc:
        with ExitStack() as ctx:
            const = ctx.enter_context(tc.tile_pool(name="const", bufs=1))
            xpool = ctx.enter_context(tc.tile_pool(name="xp", bufs=NTILES + 2))
            xtp = ctx.enter_context(tc.tile_pool(name="xtp", bufs=2, space="PSUM"))
            xts = ctx.enter_context(tc.tile_pool(name="xts", bufs=3))
            lgp = ctx.enter_context(tc.tile_pool(name="lgp", bufs=2, space="PSUM"))
            smp = ctx.enter_context(tc.tile_pool(name="smp", bufs=3))
            wpl = ctx.enter_context(tc.tile_pool(name="wpl", bufs=NTILES))
            apl = ctx.enter_context(tc.tile_pool(name="apl", bufs=3))
            atp = ctx.enter_context(tc.tile_pool(name="atp", bufs=1, space="PSUM"))
            ats = ctx.enter_context(tc.tile_pool(name="ats", bufs=3))
            outp = ctx.enter_context(tc.tile_pool(name="outp", bufs=3, space="PSUM"))
            outs = ctx.enter_context(tc.tile_pool(name="outs", bufs=3))

            wt_sb = const.tile([128, DCH, NTAPS], bf16)
            nc.sync.dma_start(wt_sb[:], wt[:, :, :])
            mk_sb = const.tile([128, NTAPS, JW], bf16)
            nc.sync.dma_start(mk_sb[:], mk.rearrange("n t j -> t n j"))
            id_sb = const.tile([128, 128], bf16)
            nc.sync.dma_start(id_sb[:], idn[:, :])

            xt = []
            for k in range(NTILES + 1):
                t = xpool.tile([TT, D], bf16, tag="x")
                nc.sync.dma_start(t[:], xs[k * TT:(k + 1) * TT, :])
                xt.append(t)

            # ---- phase 1: logits + softmax weights for every tile ----
            # Dense back-to-back PE work (transposes + matmuls) keeps the
            # HAM clock gate open; per-tile softmax runs on ACT/DVE in the
            # shadow of the PE stream.
            wsbs = []
            for i in range(NTILES):
                xc_t = xt[i + 1]
                pxt = xtp.tile([128, DCH, 128], bf16, tag="pxt")
                for k in range(DCH):
                    nc.tensor.transpose(
                        pxt[:, k, :], xc_t[:, k * 128:(k + 1) * 128], id_sb[:]
                    )
                sxt = xts.tile([128, DCH, 128], bf16, tag="sxt")
                nc.scalar.copy(sxt[:], pxt[:])
                lg = lgp.tile([128, NTAPS], f32, tag="lg")
                for k in range(DCH):
                    nc.tensor.matmul(
                        lg[:],
                        lhsT=sxt[:, k, :],
                        rhs=wt_sb[:, k, :],
                        start=(k == 0),
                        stop=(k == DCH - 1),
                    )
                # softmax over taps (free dim, 7 wide); logits are small
                # (|z| <~ 5), so skip the max-subtraction
                esb = smp.tile([128, NTAPS], f32, tag="esb")
                ssum = smp.tile([128, 1], f32, tag="ssum")
                nc.scalar.activation(
                    esb[:], lg[:], AF.Exp,
                    bias=0.0, scale=1.0, accum_out=ssum[:, 0:1],
                )
                rec = smp.tile([128, 1], f32, tag="rec")
                nc.vector.reciprocal(rec[:], ssum[:])
                wsb = wpl.tile([128, NTAPS], f32, tag="wsb")
                nc.vector.tensor_scalar(
                    out=wsb[:], in0=esb[:], scalar1=rec[:, 0:1], scalar2=None,
                    op0=OP.mult,
                )
                wsbs.append(wsb)

            # ---- phase 2: banded A, transpose, gather-matmul, store ----
            for i in range(NTILES):
                xp_t = xt[i]       # prev tile (halo for i=0)
                xc_t = xt[i + 1]   # current tile
                wsb = wsbs[i]

                # banded A build: A[t, t + 64 - o_n] = w[t, n]; init with the
                # widest tap (o=64) over the full width, then add the other
                # taps on their 128-wide bands [64-o, 192-o).
                a_t = apl.tile([128, JW], bf16, tag="a")
                nc.vector.tensor_scalar(
                    out=a_t[:], in0=mk_sb[:, 6, :], scalar1=wsb[:, 6:7],
                    scalar2=None, op0=OP.mult,
                )
                for n in range(NTAPS - 1):
                    o = OFFSETS[n]
                    bl, bh = 64 - o, 192 - o
                    nc.vector.scalar_tensor_tensor(
                        out=a_t[:, bl:bh], in0=mk_sb[:, n, bl:bh],
                        scalar=wsb[:, n:n + 1],
                        in1=a_t[:, bl:bh], op0=OP.mult, op1=OP.add,
                    )

                # A -> A.T pieces; LO piece lives at partitions [64,128) so
                # its base partition matches xp_t[64:128] in the matmul.
                pat = atp.tile([128, 2, 128], bf16, tag="pat")
                nc.tensor.transpose(pat[64:128, 0, :], a_t[:, 0:64], id_sb[:])
                nc.tensor.transpose(pat[:, 1, :], a_t[:, 64:JW], id_sb[:])
                sat = ats.tile([128, 2, 128], bf16, tag="sat")
                nc.vector.tensor_copy(sat[64:128, 0, :], pat[64:128, 0, :])
                nc.vector.tensor_copy(sat[:, 1, :], pat[:, 1, :])

                # tap-weighted gather as one banded matmul per 512-wide half
                ob = outs.tile([128, D], f32, tag="ob")
                for h in range(D // NSPLIT):
                    cs = slice(h * NSPLIT, (h + 1) * NSPLIT)
                    po = outp.tile([128, NSPLIT], f32, tag="po")
                    nc.tensor.matmul(
                        po[:], lhsT=sat[64:128, 0, :], rhs=xp_t[64:128, cs],
                        start=True, stop=False,
                    )
                    nc.tensor.matmul(
                        po[:], lhsT=sat[:, 1, :], rhs=xc_t[:, cs],
                        start=False, stop=True,
                    )
                    nc.scalar.copy(ob[:, cs], po[:])
                nc.sync.dma_start(out[i * TT:(i + 1) * TT, :], ob[:])

    nc.compile()
    return nc


def _get_program():
    if "nc" not in _prog_cache:
        _prog_cache["nc"] = _build_program()
    return _prog_cache["nc"]


def _host_inputs(x, W):
    xb = np.asarray(x).astype(BF16)
    # [1024, 7] -> [128, 8, 7]: wt[p, c, n] = W.T[c*128+p, n], matching the
    # xbar-transposed x layout (sxt[p, c, t] = x[t, c*128+p])
    wtb = np.ascontiguousarray(
        np.asarray(W, dtype=np.float32).T
        .reshape(DCH, 128, NTAPS).transpose(1, 0, 2)
    ).astype(BF16)
    mk = np.zeros((NTAPS, TT, JW), np.float32)
    for n, o in enumerate(OFFSETS):
        for t in range(TT):
            mk[n, t, t + 64 - o] = 1.0
    mk = mk.astype(BF16)
    idn = np.eye(128, dtype=np.float32).astype(BF16)
    in_maps = []
    for c in range(NCORES):
        b, h = divmod(c, 2)
        t0 = h * CHUNK
        if h == 0:
            halo = np.zeros((TT, D), BF16)
        else:
            halo = xb[b, t0 - TT:t0]
        xs = np.ascontiguousarray(
            np.concatenate([halo, xb[b, t0:t0 + CHUNK]], axis=0)
        )
        in_maps.append({"xs": xs, "wt": wtb, "mk": mk, "idn": idn})
    return in_maps


def kernel(x, W, _trace=False):
    from concourse.bass_utils import run_bass_kernel_spmd

    nc = _get_program()
    in_maps = _host_inputs(x, W)
    res = run_bass_kernel_spmd(nc, in_maps, list(range(NCORES)), trace=_trace)
    out_full = np.empty((B, T, D), np.float32)
    for c in range(NCORES):
        b, h = divmod(c, 2)
        out_full[b, h * CHUNK:(h + 1) * CHUNK] = res.results[c]["out"]
    kernel.last_results = res
    return out_full
